# revision 32
# baseline (speedup 1.0000x reference)
"""Trainium2 Bass kernel for nn_DDOpGNNUpsample (GNN message passing, cluster graphs).

Structure exploited: edges are exactly all intra-cluster ordered pairs (minus
self loops) of an 8x8 spatial grid per graph (2 graphs, 16384 nodes total).
The per-edge aggregation

    agg_i = (1/cnt_i) * sum_{j in cluster(i), j != i} ||p_i - p_j|| * x_j

collapses to a dense per-cluster matmul  aggT = x^T @ D  where
D[j,i] = ||p_i - p_j|| * inv_cnt (symmetric, ~zero diagonal).  D comes from a
rank-5 Gram matmul:  D2[i,j] = a_i . b_j  with a = [cx, cy, 1, s, 1],
b = [-2t*cx, -2t*cy, t*s, t, eps]  (coords centered per cluster,
s = cx^2+cy^2, t = inv_cnt^2), then D = sqrt(D2).  The tiny eps shift keeps
D2 strictly positive against f32 cancellation noise (bias ~1e-5, invisible),
so no relu pass is needed; padding columns are all-zero -> D2 = 0 -> D = 0.

Sharding: 128 clusters -> 16 clusters per NeuronCore (pure data parallel,
weights replicated). Each core computes out^T [8, 16*P] for its clusters.

dtypes: the Gram matmul runs in f32 (cancellation-sensitive); the encode and
aggregation matmuls run in bf16 (error ~0.3% on a mean over ~128 terms); the
N=512 projection matmuls run in float32r (single-pass fp32 PE mode).
"""
import numpy as np

B, NX, NY = 2, 8, 8
C_IN, HID, C_OUT = 8, 32, 8
ENC = 2 * HID
N_CORES = 8
N_CLUSTERS = B * NX * NY          # 128
NCL = N_CLUSTERS // N_CORES       # 16 clusters per core
WCOLS = 88                        # weight columns appended to the f32 blob
EPS = 1e-9                        # Gram diagonal shift (>> split-bf16 noise ~1e-10)


def _clusters(coords, batch):
    cx = np.clip((coords[:, 0] * NX).astype(np.int64), 0, NX - 1)
    cy = np.clip((coords[:, 1] * NY).astype(np.int64), 0, NY - 1)
    return batch * (NX * NY) + cx * NY + cy


def _build_bass_program(P, reps=1):
    import concourse.bass as bass
    import concourse.bacc as bacc
    import concourse.tile as tile
    from concourse import mybir

    f32 = mybir.dt.float32
    f32r = mybir.dt.float32r
    bf16 = mybir.dt.bfloat16
    NCOL = NCL * P
    nblks = []
    c0 = 0
    while c0 < NCOL:
        w = min(512, NCOL - c0)
        nblks.append((c0, w))
        c0 += w
    pblks = []
    p0 = 0
    while p0 < P:
        w = min(128, P - p0)
        pblks.append((p0, w))
        p0 += w
    # groups of up to 3 clusters sharing one PSUM bank for D / agg
    cgroups = [list(range(g, min(g + 3, NCL))) for g in range(0, NCL, 3)]

    nc = bacc.Bacc("TRN2", target_bir_lowering=False)
    gram16d = nc.dram_tensor("gram16", [15, 2 * NCOL], bf16, kind="ExternalInput")
    blob16 = nc.dram_tensor("blob16", [11, NCOL + 64], bf16, kind="ExternalInput")
    wts16d = nc.dram_tensor("wts16", [64, 24], bf16, kind="ExternalInput")
    out = nc.dram_tensor("out", [C_OUT, NCOL], f32, kind="ExternalOutput")

    with tile.TileContext(nc) as tc:
        with (
            tc.tile_pool(name="big", bufs=1) as big_pool,
            tc.tile_pool(name="xsb", bufs=3) as xsb_pool,
            tc.tile_pool(name="ps_x", bufs=1, space="PSUM") as ps_x,
            tc.tile_pool(name="ps_xt", bufs=2, space="PSUM") as ps_xt,
            tc.tile_pool(name="ps_d", bufs=2, space="PSUM") as ps_d,
            tc.tile_pool(name="ps_agg", bufs=1, space="PSUM") as ps_agg,
        ):
            ps_out = ps_xt
            g16 = big_pool.tile([15, 2 * NCOL], bf16, tag="gram16")
            nc.sync.dma_start(g16[:], gram16d[:])
            enc16 = big_pool.tile([11, NCOL + 64], bf16, tag="blob16")
            nc.sync.dma_start(enc16[:], blob16[:])
            wts16 = big_pool.tile([64, 24], bf16, tag="wts16")
            nc.sync.dma_start(wts16[:], wts16d[:])
            # views (split-precision Gram operands, K=15)
            cfa = g16[0:15, 0:NCOL]
            cfb = g16[0:15, NCOL:2 * NCOL]
            wrel = wts16[0:64, 0:8]
            wroot = wts16[0:64, 8:16]
            skip9 = wts16[0:9, 16:24]
            cf16 = enc16[:, 0:NCOL]
            wenc16 = enc16[:, NCOL:NCOL + 64]

            for _rep in range(reps):
                _emit_body(nc, big_pool, xsb_pool, ps_x, ps_xt, ps_d, ps_agg,
                           ps_out, cfa, cfb, wrel, wroot, skip9, cf16,
                           wenc16, out, P, NCOL, nblks, pblks, cgroups,
                           bass, mybir, f32, f32r, bf16)

    nc.compile()
    return nc


def _emit_body(nc, big_pool, xsb_pool, ps_x, ps_xt, ps_d, ps_agg, ps_out,
               cfa, cfb, wrel, wroot, skip9, cf16, wenc16, out,
               P, NCOL, nblks, pblks, cgroups, bass, mybir, f32, f32r, bf16):
    Sqrt = mybir.ActivationFunctionType.Sqrt

    aggTt = big_pool.tile([ENC, NCOL], bf16, tag="aggT")
    xTt = big_pool.tile([ENC, NCOL], bf16, tag="xT")
    dall0 = big_pool.tile([128, NCOL], bf16, tag="dall0")
    dall1 = (big_pool.tile([P - 128, NCOL], bf16, tag="dall1", name="dall1")
             if P > 128 else None)

    # --- xT = W_enc'^T applied column-wise, batched over all clusters (f32r) ---
    for (c0, w) in nblks:
        xt_ps = ps_xt.tile([ENC, 512], f32, tag="xo")
        nc.tensor.matmul(xt_ps[:, :w], wenc16, cf16[:, c0:c0 + w],
                         start=True, stop=True)
        nc.vector.tensor_copy(xTt[:, c0:c0 + w], xt_ps[:, :w])

    # --- per cluster-group: x node-major, D (Gram + sqrt), aggT ---
    for grp in cgroups:
        gw = len(grp) * P            # column width of this group's D tiles
        col0 = grp[0] * P
        d_ps0 = ps_d.tile([128, 3 * P], f32, tag="d0")
        d_ps1 = (ps_d.tile([P - 128, 3 * P], f32, tag="d1", name="d_ps1")
                 if P > 128 else None)
        agg_ps = ps_agg.tile([ENC, 3 * P], f32, tag="agg")
        xg = []
        for c in grp:
            col = c * P
            gcol = (c - grp[0]) * P
            # x node-major (bf16): both pblks into one [128, 2*ENC] psum tile
            x_ps = ps_x.tile([128, 2 * ENC], f32, tag="x")
            for bi, (p0, pw) in enumerate(pblks):
                nc.tensor.matmul(x_ps[:pw, bi * ENC:(bi + 1) * ENC],
                                 cf16[:, col + p0:col + p0 + pw], wenc16,
                                 start=True, stop=True)
            x_sb = xsb_pool.tile([128, 2 * ENC], bf16, tag="x_sb")
            nc.vector.tensor_copy(x_sb[:], x_ps[:])
            xg.append(x_sb)

            # D^2 Gram (split bf16, K=15) into the group's packed PSUM bank
            nc.tensor.matmul(d_ps0[:, gcol:gcol + P],
                             cfa[:, col:col + 128],
                             cfb[:, col:col + P], start=True, stop=True)
            if d_ps1 is not None:
                nc.tensor.matmul(d_ps1[:, gcol:gcol + P],
                                 cfa[:, col + 128:col + P],
                                 cfb[:, col:col + P], start=True, stop=True)

        # sqrt (ACT): PSUM -> SBUF bf16, one pass per packed bank
        nc.scalar.activation(dall0[:, col0:col0 + gw], d_ps0[:, :gw], Sqrt)
        if d_ps1 is not None:
            nc.scalar.activation(dall1[:, col0:col0 + gw], d_ps1[:, :gw], Sqrt)

        # aggT (bf16): per cluster, K accumulated over node blocks
        for gi, c in enumerate(grp):
            col = c * P
            gcol = gi * P
            nc.tensor.matmul(agg_ps[:, gcol:gcol + P],
                             xg[gi][0:128, 0:ENC], dall0[:, col:col + P],
                             start=True, stop=(dall1 is None))
            if dall1 is not None:
                nc.tensor.matmul(agg_ps[:, gcol:gcol + P],
                                 xg[gi][0:P - 128, ENC:2 * ENC],
                                 dall1[:, col:col + P],
                                 start=False, stop=True)
        nc.scalar.copy(aggTt[:, col0:col0 + gw], agg_ps[:, :gw])

    # --- out^T = W_rel^T aggT + W_root^T xT + skip9^T feat9 (f32r) ---
    osb = big_pool.tile([C_OUT, NCOL], f32, tag="osb")
    for (c0, w) in nblks:
        o_ps = ps_out.tile([ENC, 512], f32, tag="xo", name="o_ps")[0:C_OUT, :]
        nc.tensor.matmul(o_ps[:, :w], wrel, aggTt[:, c0:c0 + w],
                         start=True, stop=False)
        nc.tensor.matmul(o_ps[:, :w], wroot, xTt[:, c0:c0 + w],
                         start=False, stop=False)
        nc.tensor.matmul(o_ps[:, :w], skip9, cf16[0:9, c0:c0 + w],
                         start=False, stop=True)
        nc.vector.tensor_copy(osb[:, c0:c0 + w], o_ps[:, :w])
    nc.sync.dma_start(out[:], osb[:])


def _edges_match_cluster_structure(edge_index, sub, sizes):
    """Cheap host check that edge_index == all intra-cluster ordered pairs."""
    E = edge_index.shape[1]
    if E != int((sizes.astype(np.int64) * (sizes.astype(np.int64) - 1)).sum()):
        return False
    src, dst = edge_index[0].astype(np.int64), edge_index[1].astype(np.int64)
    n = sub.shape[0]
    if src.min() < 0 or src.max() >= n or dst.min() < 0 or dst.max() >= n:
        return False
    if not (sub[src] == sub[dst]).all():
        return False
    if (src == dst).any():
        return False
    pairs = src * n + dst
    return np.unique(pairs).size == E


def _reference_fallback(src_node_values, src_coords, src_batch, tgt_node_values,
                        tgt_coords, tgt_batch, edge_index, W_enc, b_enc, W_skip,
                        W_rel, b_rel, W_root):
    pos = np.concatenate([src_coords, tgt_coords], axis=0)
    vals = np.concatenate([src_node_values, tgt_node_values], axis=0)
    x = np.concatenate([vals, pos], axis=1) @ W_enc + b_enc
    N = x.shape[0]
    src_j, dst_i = edge_index[0].astype(np.int64), edge_index[1].astype(np.int64)
    w = np.linalg.norm(pos[src_j] - pos[dst_i], axis=1)
    agg = np.zeros((N, x.shape[1]), np.float32)
    np.add.at(agg, dst_i, w[:, None] * x[src_j])
    cnt = np.zeros(N, np.float32)
    np.add.at(cnt, dst_i, np.ones_like(w, np.float32))
    agg = agg / np.maximum(cnt, 1.0)[:, None]
    out = agg @ W_rel + b_rel + x @ W_root
    return (tgt_node_values @ W_skip + out[src_coords.shape[0]:]).astype(np.float32)


_PROGRAM_CACHE = {}
LAST_RESULT = None
LAST_IN_MAPS = None
LAST_P = None


def kernel(**inputs):
    inputs = {k: np.asarray(v) for k, v in inputs.items()}
    src_node_values = inputs["src_node_values"].astype(np.float32, copy=False)
    src_coords = inputs["src_coords"].astype(np.float32, copy=False)
    tgt_node_values = inputs["tgt_node_values"].astype(np.float32, copy=False)
    tgt_coords = inputs["tgt_coords"].astype(np.float32, copy=False)
    W_enc = inputs["W_enc"].astype(np.float32, copy=False)
    b_enc = inputs["b_enc"].astype(np.float32, copy=False)
    W_skip = inputs["W_skip"].astype(np.float32, copy=False)
    W_rel = inputs["W_rel"].astype(np.float32, copy=False)
    b_rel = inputs["b_rel"].astype(np.float32, copy=False)
    W_root = inputs["W_root"].astype(np.float32, copy=False)
    edge_index = inputs["edge_index"]

    pos = np.concatenate([src_coords, tgt_coords], axis=0)
    vals = np.concatenate([src_node_values, tgt_node_values], axis=0)
    batch = np.concatenate([inputs["src_batch"], inputs["tgt_batch"]]).astype(np.int64)
    N = pos.shape[0]
    N_SRC = src_coords.shape[0]

    sub = _clusters(pos, batch)
    sizes = np.bincount(sub, minlength=N_CLUSTERS)
    if len(sizes) != N_CLUSTERS or not _edges_match_cluster_structure(
            edge_index, sub, sizes):
        return _reference_fallback(
            src_node_values, src_coords, inputs["src_batch"], tgt_node_values,
            tgt_coords, inputs["tgt_batch"], edge_index, W_enc, b_enc, W_skip,
            W_rel, b_rel, W_root)

    order = np.argsort(sub, kind="stable")
    starts = np.zeros(N_CLUSTERS + 1, np.int64)
    np.cumsum(sizes, out=starts[1:])
    P = max(160, int(np.ceil(max(sizes.max(), 1) / 32) * 32))
    NCOL = NCL * P

    import ml_dtypes
    bf16 = ml_dtypes.bfloat16

    # packed weights (shared across cores)
    W_enc11 = np.ascontiguousarray(
        np.concatenate([W_enc[0:C_IN], b_enc[None, :], W_enc[C_IN:C_IN + 2]], axis=0))
    skip9 = np.ascontiguousarray(np.concatenate([W_skip, b_rel[None, :]], axis=0))

    def split16(v):
        h = v.astype(bf16)
        l = (v - h.astype(np.float32)).astype(bf16)
        return h, l

    in_maps = []
    for core in range(N_CORES):
        feat = np.zeros((11, NCOL), np.float32)
        A = np.zeros((5, NCOL), np.float32)   # virtual a rows
        Bv = np.zeros((5, NCOL), np.float32)  # virtual b rows
        for c in range(NCL):
            g = core * NCL + c
            n = int(sizes[g])
            idx = order[starts[g]:starts[g + 1]]
            col = c * P
            feat[0:8, col:col + n] = vals[idx].T
            feat[8, col:col + n] = 1.0
            feat[9, col:col + n] = pos[idx, 0]
            feat[10, col:col + n] = pos[idx, 1]
            if n > 0:
                mx = pos[idx, 0].mean(dtype=np.float64).astype(np.float32)
                my = pos[idx, 1].mean(dtype=np.float64).astype(np.float32)
                cx = pos[idx, 0] - mx
                cy = pos[idx, 1] - my
            else:
                cx = cy = np.zeros(0, np.float32)
            s = (cx * cx + cy * cy).astype(np.float32)
            t = np.float32(1.0 / max(n - 1, 1)) ** 2
            A[0, col:col + n] = cx
            A[1, col:col + n] = cy
            A[2, col:col + n] = 1.0
            A[3, col:col + n] = s
            A[4, col:col + n] = 1.0
            Bv[0, col:col + n] = -2.0 * t * cx
            Bv[1, col:col + n] = -2.0 * t * cy
            Bv[2, col:col + n] = t * s
            Bv[3, col:col + n] = t
            Bv[4, col:col + n] = EPS
        ah, al = split16(A)
        bh, bl = split16(Bv)
        gram16 = np.zeros((15, 2 * NCOL), bf16)
        gram16[0:5, 0:NCOL] = ah; gram16[5:10, 0:NCOL] = ah
        gram16[10:15, 0:NCOL] = al
        gram16[0:5, NCOL:] = bh; gram16[5:10, NCOL:] = bl
        gram16[10:15, NCOL:] = bh
        blob16 = np.zeros((11, NCOL + 64), bf16)
        blob16[:, 0:NCOL] = feat.astype(bf16)
        blob16[:, NCOL:NCOL + 64] = W_enc11.astype(bf16)
        wts16 = np.zeros((64, 24), bf16)
        wts16[0:64, 0:8] = W_rel.astype(bf16)
        wts16[0:64, 8:16] = W_root.astype(bf16)
        wts16[0:9, 16:24] = skip9.astype(bf16)
        in_maps.append({"gram16": gram16, "blob16": blob16, "wts16": wts16})

    from concourse import bass_utils
    global LAST_IN_MAPS, LAST_P
    LAST_IN_MAPS, LAST_P = in_maps, P
    if P not in _PROGRAM_CACHE:
        _PROGRAM_CACHE[P] = _build_bass_program(P)
    nc = _PROGRAM_CACHE[P]
    import os
    trace = bool(os.environ.get("KERNEL_PROFILE"))
    if trace:
        try:
            from antenv.axon_hooks import get_axon_ntff_profile_hook  # noqa: F401
        except ImportError:
            trace = False
    res = bass_utils.run_bass_kernel_spmd(
        nc, in_maps, core_ids=list(range(N_CORES)), trace=trace)
    global LAST_RESULT
    LAST_RESULT = res
    results = res.results

    out_full = np.zeros((N, C_OUT), np.float32)
    for core in range(N_CORES):
        outT = results[core]["out"]          # [8, NCOL]
        for c in range(NCL):
            g = core * NCL + c
            n = int(sizes[g])
            idx = order[starts[g]:starts[g + 1]]
            out_full[idx] = outT[:, c * P:c * P + n].T
    return out_full[N_SRC:]


# revision 49
# speedup vs baseline: 90.4726x; 90.4726x over previous
"""Trainium2 Bass kernel for nn_DDOpGNNUpsample (GNN message passing, cluster graphs).

Structure exploited: edges are exactly all intra-cluster ordered pairs (minus
self loops) of an 8x8 spatial grid per graph (2 graphs, 16384 nodes total).
The per-edge aggregation

    agg_i = (1/cnt_i) * sum_{j in cluster(i), j != i} ||p_i - p_j|| * x_j

collapses to a dense per-cluster matmul  aggT = x^T @ D  where
D[j,i] = ||p_i - p_j|| * inv_cnt (symmetric, ~zero diagonal).  D comes from a
rank-5 virtual Gram:  D2[i,j] = a_i . b_j  with a = [cx, cy, 1, s, 1],
b = [-2t*cx, -2t*cy, t*s, t, eps]  (coords centered per cluster,
s = cx^2+cy^2, t = inv_cnt^2), then D = sqrt(D2).  On device this runs as a
K=15 split-precision bf16 matmul (hi/lo Dekker split of both sides: hh+hl+lh
terms), giving ~f32 accuracy at bf16 speed.  The tiny eps shift keeps D2
strictly positive against the ~1e-10 split noise (bias < 1e-5 per weight), so
no relu pass is needed; padding columns are all-zero -> D2 = 0 -> D = 0.

Sharding: 128 clusters -> 16 clusters per NeuronCore (pure data parallel,
weights replicated). Each core computes out^T [8, 16*P] for its clusters.
The x @ W_root and skip terms are folded on the host into one fused weight
W_comb = W_enc' @ W_root + [W_skip; b_rel; 0; 0], removing the xT stage.
"""
import numpy as np

B, NX, NY = 2, 8, 8
C_IN, HID, C_OUT = 8, 32, 8
ENC = 2 * HID
N_CORES = 8
N_CLUSTERS = B * NX * NY          # 128
NCL = N_CLUSTERS // N_CORES       # 16 clusters per core
EPS = 1e-9                        # Gram diagonal shift (>> split-bf16 noise ~1e-10)


def _clusters(coords, batch):
    cx = np.clip((coords[:, 0] * NX).astype(np.int64), 0, NX - 1)
    cy = np.clip((coords[:, 1] * NY).astype(np.int64), 0, NY - 1)
    return batch * (NX * NY) + cx * NY + cy


def _build_bass_program(P, reps=1):
    import concourse.bass as bass
    import concourse.bacc as bacc
    import concourse.tile as tile
    from concourse import mybir

    f32 = mybir.dt.float32
    f32r = mybir.dt.float32r
    bf16 = mybir.dt.bfloat16
    NCOL = NCL * P
    nblks = []
    c0 = 0
    while c0 < NCOL:
        w = min(3 * P, NCOL - c0)    # group-aligned out blocks
        nblks.append((c0, w))
        c0 += w
    pblks = []
    p0 = 0
    while p0 < P:
        w = min(128, P - p0)
        pblks.append((p0, w))
        p0 += w
    # groups of up to 3 clusters sharing one PSUM bank for D / agg
    cgroups = [list(range(g, min(g + 3, NCL))) for g in range(0, NCL, 3)]

    nc = bacc.Bacc("TRN2", target_bir_lowering=False)
    gram16d = nc.dram_tensor("gram16", [15, 2 * NCOL], bf16, kind="ExternalInput")
    blob16 = nc.dram_tensor("blob16", [11, NCOL + 64], bf16, kind="ExternalInput")
    wts16d = nc.dram_tensor("wts16", [64, 16], bf16, kind="ExternalInput")
    out = nc.dram_tensor("out", [C_OUT, NCOL], f32, kind="ExternalOutput")

    with tile.TileContext(nc) as tc:
        with (
            tc.tile_pool(name="big", bufs=1) as big_pool,
            tc.tile_pool(name="xsb", bufs=3) as xsb_pool,
            tc.tile_pool(name="ps_x", bufs=2, space="PSUM") as ps_x,
            tc.tile_pool(name="ps_xt", bufs=1, space="PSUM") as ps_xt,
            tc.tile_pool(name="ps_d", bufs=2, space="PSUM") as ps_d,
            tc.tile_pool(name="ps_agg", bufs=1, space="PSUM") as ps_agg,
        ):
            ps_out = ps_xt
            g16 = big_pool.tile([15, 2 * NCOL], bf16, tag="gram16")
            nc.sync.dma_start(g16[:], gram16d[:])
            enc16 = big_pool.tile([11, NCOL + 64], bf16, tag="blob16")
            nc.sync.dma_start(enc16[:], blob16[:])
            wts16 = big_pool.tile([64, 16], bf16, tag="wts16")
            nc.sync.dma_start(wts16[:], wts16d[:])
            # views (split-precision Gram operands, K=15)
            cfa = g16[0:15, 0:NCOL]
            cfb = g16[0:15, NCOL:2 * NCOL]
            wrel = wts16[0:64, 0:8]
            wcomb = wts16[0:11, 8:16]
            cf16 = enc16[:, 0:NCOL]
            wenc16 = enc16[:, NCOL:NCOL + 64]

            for _rep in range(reps):
                _emit_body(nc, big_pool, xsb_pool, ps_x, ps_xt, ps_d, ps_agg,
                           ps_out, cfa, cfb, wrel, wcomb, cf16,
                           wenc16, out, P, NCOL, nblks, pblks, cgroups,
                           bass, mybir, f32, f32r, bf16)

    nc.compile()
    return nc


def _emit_body(nc, big_pool, xsb_pool, ps_x, ps_xt, ps_d, ps_agg, ps_out,
               cfa, cfb, wrel, wcomb, cf16, wenc16, out,
               P, NCOL, nblks, pblks, cgroups, bass, mybir, f32, f32r, bf16):
    abl = globals().get("ABLATE", frozenset())
    Sqrt = mybir.ActivationFunctionType.Sqrt
    vpack = (P - 128 == 32)          # vertical 3-packing of the 32-row strips

    aggTt = big_pool.tile([ENC, NCOL], bf16, tag="aggT")
    dall0 = big_pool.tile([128, NCOL], bf16, tag="dall0")
    if P > 128:
        dall1 = big_pool.tile([96 if vpack else P - 128,
                               len(cgroups) * P if vpack else NCOL],
                              bf16, tag="dall1", name="dall1")
    else:
        dall1 = None

    # --- per cluster-group: x node-major, D (Gram + sqrt), aggT ---
    for gidx, grp in enumerate(cgroups):
        gw = len(grp) * P            # column width of this group's D tiles
        col0 = grp[0] * P
        d_ps0 = ps_d.tile([128, 3 * P], f32, tag="d0")
        if P > 128:
            d_ps1 = ps_d.tile([96, P] if vpack else [P - 128, 3 * P],
                              f32, tag="d1", name="d_ps1")
        else:
            d_ps1 = None
        agg_ps = ps_agg.tile([ENC, 3 * P], f32, tag="agg")
        # x node-major (bf16): all clusters of the group share one PSUM bank
        # (cluster gi: blk0 at [0:128, gi*128 : gi*128+64], blk1 at
        #  [32*gi : 32*gi+32, gi*128+64 : (gi+1)*128]); one copy per group
        x_ps = ps_x.tile([128, 3 * 2 * ENC], f32, tag="x")
        x_sb = xsb_pool.tile([128, 3 * 2 * ENC], bf16, tag="x_sb")
        if "x" not in abl:
            for c in grp:
                col = c * P
                gi0 = c - grp[0]
                for bi, (p0, pw) in enumerate(pblks):
                    r0 = 32 * gi0 if (bi == 1 and vpack) else 0
                    cc = gi0 * 2 * ENC + bi * ENC
                    nc.tensor.matmul(x_ps[r0:r0 + pw, cc:cc + ENC],
                                     cf16[:, col + p0:col + p0 + pw], wenc16,
                                     start=True, stop=True)
            nc.vector.tensor_copy(x_sb[:, 0:len(grp) * 2 * ENC],
                                  x_ps[:, 0:len(grp) * 2 * ENC])
        xg = [x_sb[:, gi * 2 * ENC:(gi + 1) * 2 * ENC] for gi in range(len(grp))]
        for c in grp:
            col = c * P
            gcol = (c - grp[0]) * P

            if "d" in abl:
                continue
            # D^2 Gram (split bf16, K=15) into the group's packed PSUM bank
            nc.tensor.matmul(d_ps0[:, gcol:gcol + P],
                             cfa[:, col:col + 128],
                             cfb[:, col:col + P], start=True, stop=True)
            if d_ps1 is not None:
                if vpack:
                    gi = c - grp[0]
                    nc.tensor.matmul(d_ps1[32 * gi:32 * gi + 32, 0:P],
                                     cfa[:, col + 128:col + P],
                                     cfb[:, col:col + P], start=True, stop=True)
                else:
                    nc.tensor.matmul(d_ps1[:, gcol:gcol + P],
                                     cfa[:, col + 128:col + P],
                                     cfb[:, col:col + P], start=True, stop=True)

        # sqrt (ACT): PSUM -> SBUF bf16, one pass per packed bank
        if "d" in abl or "sqrt" in abl:
            pass
        else:
            nc.scalar.activation(dall0[:, col0:col0 + gw], d_ps0[:, :gw], Sqrt)
        if d_ps1 is not None and "d" not in abl and "sqrt" not in abl:
            if vpack:
                nr = 32 * len(grp)
                nc.scalar.activation(dall1[0:nr, gidx * P:gidx * P + P],
                                     d_ps1[0:nr, 0:P], Sqrt)
            else:
                nc.scalar.activation(dall1[:, col0:col0 + gw], d_ps1[:, :gw], Sqrt)

        # aggT (bf16): per cluster, K accumulated over node blocks
        if "agg" in abl:
            continue
        for gi, c in enumerate(grp):
            col = c * P
            gcol = gi * P
            nc.tensor.matmul(agg_ps[:, gcol:gcol + P],
                             xg[gi][0:128, 0:ENC], dall0[:, col:col + P],
                             start=True, stop=(dall1 is None))
            if dall1 is not None:
                if vpack:
                    rhs1 = dall1[32 * gi:32 * gi + 32, gidx * P:gidx * P + P]
                    lhs1 = xg[gi][32 * gi:32 * gi + 32, ENC:2 * ENC]
                else:
                    rhs1 = dall1[:, col:col + P]
                    lhs1 = xg[gi][0:P - 128, ENC:2 * ENC]
                nc.tensor.matmul(agg_ps[:, gcol:gcol + P], lhs1, rhs1,
                                 start=False, stop=True)
        if gidx % 2 == 0:
            nc.scalar.copy(aggTt[:, col0:col0 + gw], agg_ps[:, :gw])
        else:
            nc.vector.tensor_copy(aggTt[:, col0:col0 + gw], agg_ps[:, :gw])

    # --- out^T = W_rel^T aggT + W_comb^T feat11  (W_comb = W_enc' W_root
    #     + [W_skip; b_rel; 0; 0], fused on host) ---
    osb = big_pool.tile([C_OUT, NCOL], f32, tag="osb")
    if "out" in abl:
        nc.sync.dma_start(out[:], osb[:])
        return
    for (c0, w) in nblks:
        o_ps = ps_out.tile([ENC, 512], f32, tag="xo", name="o_ps")[0:C_OUT, :]
        nc.tensor.matmul(o_ps[:, :w], wrel, aggTt[:, c0:c0 + w],
                         start=True, stop=False)
        nc.tensor.matmul(o_ps[:, :w], wcomb, cf16[:, c0:c0 + w],
                         start=False, stop=True)
        nc.vector.tensor_copy(osb[:, c0:c0 + w], o_ps[:, :w])
        nc.sync.dma_start(out[:, c0:c0 + w], osb[:, c0:c0 + w])


def _edges_match_cluster_structure(edge_index, sub, sizes):
    """Cheap host check that edge_index == all intra-cluster ordered pairs."""
    E = edge_index.shape[1]
    if E != int((sizes.astype(np.int64) * (sizes.astype(np.int64) - 1)).sum()):
        return False
    src, dst = edge_index[0].astype(np.int64), edge_index[1].astype(np.int64)
    n = sub.shape[0]
    if src.min() < 0 or src.max() >= n or dst.min() < 0 or dst.max() >= n:
        return False
    if not (sub[src] == sub[dst]).all():
        return False
    if (src == dst).any():
        return False
    pairs = src * n + dst
    return np.unique(pairs).size == E


def _reference_fallback(src_node_values, src_coords, src_batch, tgt_node_values,
                        tgt_coords, tgt_batch, edge_index, W_enc, b_enc, W_skip,
                        W_rel, b_rel, W_root):
    pos = np.concatenate([src_coords, tgt_coords], axis=0)
    vals = np.concatenate([src_node_values, tgt_node_values], axis=0)
    x = np.concatenate([vals, pos], axis=1) @ W_enc + b_enc
    N = x.shape[0]
    src_j, dst_i = edge_index[0].astype(np.int64), edge_index[1].astype(np.int64)
    w = np.linalg.norm(pos[src_j] - pos[dst_i], axis=1)
    agg = np.zeros((N, x.shape[1]), np.float32)
    np.add.at(agg, dst_i, w[:, None] * x[src_j])
    cnt = np.zeros(N, np.float32)
    np.add.at(cnt, dst_i, np.ones_like(w, np.float32))
    agg = agg / np.maximum(cnt, 1.0)[:, None]
    out = agg @ W_rel + b_rel + x @ W_root
    return (tgt_node_values @ W_skip + out[src_coords.shape[0]:]).astype(np.float32)


_PROGRAM_CACHE = {}
ABLATE = frozenset()
LAST_RESULT = None
LAST_IN_MAPS = None
LAST_P = None


def kernel(**inputs):
    inputs = {k: np.asarray(v) for k, v in inputs.items()}
    src_node_values = inputs["src_node_values"].astype(np.float32, copy=False)
    src_coords = inputs["src_coords"].astype(np.float32, copy=False)
    tgt_node_values = inputs["tgt_node_values"].astype(np.float32, copy=False)
    tgt_coords = inputs["tgt_coords"].astype(np.float32, copy=False)
    W_enc = inputs["W_enc"].astype(np.float32, copy=False)
    b_enc = inputs["b_enc"].astype(np.float32, copy=False)
    W_skip = inputs["W_skip"].astype(np.float32, copy=False)
    W_rel = inputs["W_rel"].astype(np.float32, copy=False)
    b_rel = inputs["b_rel"].astype(np.float32, copy=False)
    W_root = inputs["W_root"].astype(np.float32, copy=False)
    edge_index = inputs["edge_index"]

    pos = np.concatenate([src_coords, tgt_coords], axis=0)
    vals = np.concatenate([src_node_values, tgt_node_values], axis=0)
    batch = np.concatenate([inputs["src_batch"], inputs["tgt_batch"]]).astype(np.int64)
    N = pos.shape[0]
    N_SRC = src_coords.shape[0]

    sub = _clusters(pos, batch)
    sizes = np.bincount(sub, minlength=N_CLUSTERS)
    if len(sizes) != N_CLUSTERS or not _edges_match_cluster_structure(
            edge_index, sub, sizes):
        return _reference_fallback(
            src_node_values, src_coords, inputs["src_batch"], tgt_node_values,
            tgt_coords, inputs["tgt_batch"], edge_index, W_enc, b_enc, W_skip,
            W_rel, b_rel, W_root)

    order = np.argsort(sub, kind="stable")
    starts = np.zeros(N_CLUSTERS + 1, np.int64)
    np.cumsum(sizes, out=starts[1:])
    P = max(160, int(np.ceil(max(sizes.max(), 1) / 32) * 32))
    NCOL = NCL * P

    import ml_dtypes
    bf16 = ml_dtypes.bfloat16

    # packed weights (shared across cores)
    W_enc11 = np.ascontiguousarray(
        np.concatenate([W_enc[0:C_IN], b_enc[None, :], W_enc[C_IN:C_IN + 2]], axis=0))
    skip9 = np.ascontiguousarray(np.concatenate([W_skip, b_rel[None, :]], axis=0))

    def split16(v):
        h = v.astype(bf16)
        l = (v - h.astype(np.float32)).astype(bf16)
        return h, l

    in_maps = []
    for core in range(N_CORES):
        feat = np.zeros((11, NCOL), np.float32)
        A = np.zeros((5, NCOL), np.float32)   # virtual a rows
        Bv = np.zeros((5, NCOL), np.float32)  # virtual b rows
        for c in range(NCL):
            g = core * NCL + c
            n = int(sizes[g])
            idx = order[starts[g]:starts[g + 1]]
            col = c * P
            feat[0:8, col:col + n] = vals[idx].T
            feat[8, col:col + n] = 1.0
            feat[9, col:col + n] = pos[idx, 0]
            feat[10, col:col + n] = pos[idx, 1]
            if n > 0:
                mx = pos[idx, 0].mean(dtype=np.float64).astype(np.float32)
                my = pos[idx, 1].mean(dtype=np.float64).astype(np.float32)
                cx = pos[idx, 0] - mx
                cy = pos[idx, 1] - my
            else:
                cx = cy = np.zeros(0, np.float32)
            s = (cx * cx + cy * cy).astype(np.float32)
            t = np.float32(1.0 / max(n - 1, 1)) ** 2
            A[0, col:col + n] = cx
            A[1, col:col + n] = cy
            A[2, col:col + n] = 1.0
            A[3, col:col + n] = s
            A[4, col:col + n] = 1.0
            Bv[0, col:col + n] = -2.0 * t * cx
            Bv[1, col:col + n] = -2.0 * t * cy
            Bv[2, col:col + n] = t * s
            Bv[3, col:col + n] = t
            Bv[4, col:col + n] = EPS
        ah, al = split16(A)
        bh, bl = split16(Bv)
        gram16 = np.zeros((15, 2 * NCOL), bf16)
        gram16[0:5, 0:NCOL] = ah; gram16[5:10, 0:NCOL] = ah
        gram16[10:15, 0:NCOL] = al
        gram16[0:5, NCOL:] = bh; gram16[5:10, NCOL:] = bl
        gram16[10:15, NCOL:] = bh
        blob16 = np.zeros((11, NCOL + 64), bf16)
        blob16[:, 0:NCOL] = feat.astype(bf16)
        blob16[:, NCOL:NCOL + 64] = W_enc11.astype(bf16)
        W_comb = (W_enc11.astype(np.float64) @ W_root.astype(np.float64))
        W_comb[0:9] += skip9.astype(np.float64)
        wts16 = np.zeros((64, 16), bf16)
        wts16[0:64, 0:8] = W_rel.astype(bf16)
        wts16[0:11, 8:16] = W_comb.astype(np.float32).astype(bf16)
        in_maps.append({"gram16": gram16, "blob16": blob16, "wts16": wts16})

    from concourse import bass_utils
    global LAST_IN_MAPS, LAST_P
    LAST_IN_MAPS, LAST_P = in_maps, P
    if P not in _PROGRAM_CACHE:
        _PROGRAM_CACHE[P] = _build_bass_program(P)
    nc = _PROGRAM_CACHE[P]
    import os
    trace = bool(os.environ.get("KERNEL_PROFILE"))
    if trace:
        try:
            from antenv.axon_hooks import get_axon_ntff_profile_hook  # noqa: F401
        except ImportError:
            trace = False
    res = bass_utils.run_bass_kernel_spmd(
        nc, in_maps, core_ids=list(range(N_CORES)), trace=trace)
    global LAST_RESULT
    LAST_RESULT = res
    results = res.results

    out_full = np.zeros((N, C_OUT), np.float32)
    for core in range(N_CORES):
        outT = results[core]["out"]          # [8, NCOL]
        for c in range(NCL):
            g = core * NCL + c
            n = int(sizes[g])
            idx = order[starts[g]:starts[g + 1]]
            out_full[idx] = outT[:, c * P:c * P + n].T
    return out_full[N_SRC:]
